# revision 10
# baseline (speedup 1.0000x reference)
"""Trainium2 Bass kernel for nn_CustomParameterTransform (scatter_memory).

Reference semantics: coord_v [256, 30] holds 10 (x, y, mass) triplets per
sample. Each triplet maps to integer grid indices (x_i, y_i, m_i); a one-hot
volume z [B, 16, 128, 128] is scattered (z[b, m, y, x] = 1) and the output is
concat(1-z, z) over the channel axis -> [256, 32, 128, 128] f32 (512 MB).

Strategy (8 NeuronCores, batch-sharded, no cross-core comm):
  - The output is almost entirely constant: the first 16 channels are 1.0
    except at scatter points, the last 16 are 0.0 except at scatter points.
  - Per core (32 samples, 64 MB slab): fill the slab from constant SBUF
    tiles with large DMAs (write-only HBM traffic; ~425 GB/s sustained =
    16 SDMA engines x ~26.6 GB/s), then fix up the 640 scatter points with
    indirect (scatter) DMAs on the gpsimd/SWDGE queue.
  - Indices are computed on the host with the exact same jax ops as the
    reference (bit-identical floor/log10 behavior) and passed per-core as a
    [128, 6] int32 tensor of flat element offsets.

Trace-driven tuning (what each piece buys):
  - First memsets on the vector engine (gpsimd takes ~7 us to wake after
    the NEFF startup barrier; vector is ready at ~4.8 us) -> first fill
    at ~5.6 us instead of 8.4 us.
  - Fills run in ascending sample order; scatter columns for samples 0-29
    depend on fills that complete mid-stream, so those scatters fully
    overlap the fill phase. Only the last fill (samples 30-31) gates a
    scatter: a small 64-row column (fast SWDGE dispatch) -> post-fill tail
    ~2.5 us instead of ~4 us for a 128-row column gated on everything.
  - Light drain/barrier epilogue (see _light_drain_and_barrier). The
    remaining exit cost (~7 us: all-engine barrier + a NEFF-level sweep
    zeroing all 253 event semaphores + exit barrier) is emitted by the
    NEFF toolchain, not bass, and is not reachable from kernel code.
"""

import numpy as np

B = 256
NSRC = 10
NMC = 16
L = 128
NCORES = 8
BL = B // NCORES          # 32 samples per core
PLANE = L * L             # 16384
HALF = NMC * PLANE        # 262144 elements per half-slab
SLAB = 2 * HALF           # 524288 elements per sample
OUT_ELEMS = BL * SLAB     # 16777216 per core (64 MB)

N_SCATTER_COLS = 9        # cols 0-6: 4 samples each; col 7: s28-29; col 8: s30-31

_CACHE = {}


def _build_nc():
    import concourse.bass as bass
    import concourse.tile as tile
    from concourse import bacc, mybir
    from concourse.tile_rust import add_dep_helper

    import types as _types
    from concourse.vector_clock import ScopedClock

    nc = bacc.Bacc("TRN2", target_bir_lowering=False, debug=False,
                   num_devices=NCORES)

    def _light_drain_and_barrier(self, tick_clock, wait_clock):
        """Replaces TileContext._drain_and_barrier for this kernel. The
        stock epilogue is drain + two all-engine EVSEM butterfly barriers
        around the sem clear. Requirements at kernel end are: (1) all DMA
        completions observed, (2) sems cleared for NEFF re-execution,
        (3) the clear happens after every engine's last sem use. (1) is
        the sync drain's global-clock waits; (3) is a counting-sem join
        (sync arrives only after the drain, so join>=4 implies all DMA
        done); (2) is the ranged clear. The second barrier is
        unnecessary: a re-execution cannot start until every engine --
        including the clearing gpsimd -- has ended."""
        nc_ = self.nc
        drain_inst = nc_.sync.drain()
        wait_clock.add_sem_waits(
            drain_inst.ins, ScopedClock({None: tick_clock.global_clock}))
        join = nc_.alloc_semaphore("tail_join")
        for eng in nc_.engines.values():
            if eng is not nc_.gpsimd:
                eng.sem_inc(join, 1)
        n_other = len(nc_.engines) - 1
        nc_.gpsimd.wait_ge(join, n_other)
        popped = nc_._tile_sem_poison_stack.pop()
        assert popped is self._sem_poison
        sems = list(self.sems.allocated().values())
        nc_.clear_and_free_semaphores(sems + [join])

    offs = nc.dram_tensor("offs", [128, N_SCATTER_COLS], mybir.dt.int32,
                          kind="ExternalInput").ap()
    out = nc.dram_tensor("out", [OUT_ELEMS], mybir.dt.float32,
                         kind="ExternalOutput").ap()

    with tile.TileContext(nc) as tc:
        tc._drain_and_barrier = _types.MethodType(_light_drain_and_barrier, tc)
        with tc.tile_pool(name="src", bufs=1) as src_pool, \
             tc.tile_pool(name="small", bufs=1) as small_pool:
            # Constant source tiles. Memset cost scales with the free-dim
            # cols (128 lanes run in parallel), so big tiles are split
            # column-wise between vector and gpsimd. Vector handles
            # everything needed early (it wakes ~2 us before gpsimd).
            ones_mini = src_pool.tile([128, 1024], mybir.dt.float32)
            zeros_mini = src_pool.tile([128, 1024], mybir.dt.float32)
            nc.vector.memset(ones_mini[:, :], 1.0)
            nc.vector.memset(zeros_mini[:, :], 0.0)
            # combo: one full slab ([128, 4096]; DMA iterates partition-
            # major, so partitions 0-63 are the ones half, 64-127 zeros).
            combo_t = src_pool.tile([128, 4096], mybir.dt.float32)
            nc.vector.memset(combo_t[0:64, 0:2048], 1.0)
            nc.vector.memset(combo_t[64:128, 0:2048], 0.0)
            nc.gpsimd.memset(combo_t[0:64, 2048:4096], 1.0)
            nc.gpsimd.memset(combo_t[64:128, 2048:4096], 0.0)
            # mega: two slabs ([128, 8192]; slab = 64 partitions, ones iff
            # p%64 < 32). Feeds samples 6-29 as 4 MB fills.
            mega_t = src_pool.tile([128, 8192], mybir.dt.float32)
            for lo, hi, v in ((0, 32, 1.0), (32, 64, 0.0),
                              (64, 96, 1.0), (96, 128, 0.0)):
                nc.vector.memset(mega_t[lo:hi, 0:4096], v)
                nc.gpsimd.memset(mega_t[lo:hi, 4096:8192], v)

            # Scatter offsets: [128, 9] int32 flat element indices.
            # Columns are ALIGNED TO FILL ORDER so each column's fill deps
            # complete early in the stream and the scatter hides under the
            # remaining fills. (A column spanning 13 samples would gate on
            # the latest of 7 fills -> a serialized gpsimd dispatch pileup
            # at the stream tail, ~+6 us.)
            #   col j in 0..6: samples 4j..4j+3; rows 0-39 ones-half
            #                  writes (0.0), rows 40-79 z-half writes (1.0)
            #   col 7: samples 28-29; rows 0-19 ones (+12 dup-pad rows),
            #          rows 32-51 z
            #   col 8: samples 30-31 (the final fills); same row layout
            # offs loads on the scalar HWDGE queue before its fills (no
            # deps, so Tile lets it lead); vals memsets ride on vector
            # after its big memsets. Engine ops must start at a partition
            # multiple of 32, hence the overwrite at rows 32:40.
            offs_t = small_pool.tile([128, N_SCATTER_COLS], mybir.dt.int32)
            vals_t = small_pool.tile([128, N_SCATTER_COLS], mybir.dt.float32)
            nc.vector.memset(vals_t[0:32, 0:7], 0.0)
            nc.vector.memset(vals_t[32:64, 0:7], 1.0)
            nc.vector.memset(vals_t[32:40, 0:7], 0.0)
            nc.vector.memset(vals_t[64:96, 0:7], 1.0)
            nc.vector.memset(vals_t[0:32, 7:9], 0.0)
            nc.vector.memset(vals_t[32:64, 7:9], 1.0)

            MINI = 131072  # elements per mini fill (512 KB)
            ones_fills = {}   # sample -> list of fills covering its ones half
            zeros_fills = {}  # sample -> list of fills covering its zeros half
            fill_seq = {"sync": 0, "scalar": 0}

            # Samples 0-1 from the minis (ready first).
            for s in (0, 1):
                e_ones = nc.sync if s == 0 else nc.scalar
                e_zeros = nc.scalar if s == 0 else nc.sync
                ones_fills[s] = [
                    e_ones.dma_start(
                        out[s * SLAB + k * MINI:s * SLAB + (k + 1) * MINI],
                        ones_mini[:, :])
                    for k in range(2)]
                zeros_fills[s] = [
                    e_zeros.dma_start(
                        out[s * SLAB + HALF + k * MINI:
                            s * SLAB + HALF + (k + 1) * MINI],
                        zeros_mini[:, :])
                    for k in range(2)]
            # offs load sits on the scalar queue here: after the first
            # mini fills (so it doesn't delay the stream start), done by
            # ~8 us, way before the first scatter needs it (~70 us).
            offs_fill = nc.scalar.dma_start(offs_t[:, :], offs[:, :])
            # Samples 2-5 from combo (2 MB fills).
            for s in range(2, 6):
                eng = nc.sync if s % 2 == 0 else nc.scalar
                f = eng.dma_start(out[s * SLAB:(s + 1) * SLAB], combo_t[:, :])
                ones_fills[s] = [f]
                zeros_fills[s] = [f]
            # Samples 6-29 from mega (4 MB pair fills), ascending, so the
            # scatter columns' fill deps complete early in the stream.
            for s in range(6, 30, 2):
                eng = nc.sync if (s // 2) % 2 == 0 else nc.scalar
                f = eng.dma_start(out[s * SLAB:(s + 2) * SLAB], mega_t[:, :])
                for ss in (s, s + 1):
                    ones_fills[ss] = [f]
                    zeros_fills[ss] = [f]
            # Samples 30-31 last, one 2 MB fill per queue (balances both
            # queues at 32 MB) from mega's two slab halves.
            f30 = nc.sync.dma_start(out[30 * SLAB:31 * SLAB], mega_t[0:64, :])
            f31 = nc.scalar.dma_start(out[31 * SLAB:32 * SLAB],
                                      mega_t[64:128, :])
            ones_fills[30] = [f30]
            zeros_fills[30] = [f30]
            ones_fills[31] = [f31]
            zeros_fills[31] = [f31]

            # Which sample-fills each scatter column touches.
            def deps(lo, hi):
                seen = {}
                for s in range(lo, hi):
                    for f in ones_fills[s] + zeros_fills[s]:
                        seen[id(f)] = f
                return list(seen.values())
            col_specs = [(slice(0, 80), deps(4 * j, 4 * j + 4))
                         for j in range(7)]
            col_specs.append((slice(0, 52), deps(28, 30)))
            col_specs.append((slice(0, 52), deps(30, 32)))

            # Narrow declared out AP ([1, 1] at offset 0): the real write
            # addresses come from the offset tensor; a full-tensor AP would
            # make Tile serialize every scatter behind every fill (WAW), and
            # the explicit col_deps edges below provide the true ordering.
            out2d = out[0:1].unsqueeze(1)
            for j, (rows, fl_deps) in enumerate(col_specs):
                sc = nc.gpsimd.indirect_dma_start(
                    out=out2d,
                    out_offset=bass.IndirectOffsetOnAxis(
                        ap=offs_t[rows, j:j + 1], axis=0),
                    in_=vals_t[rows, j:j + 1],
                    in_offset=None,
                )
                for fl in fl_deps:
                    add_dep_helper(sc.ins, fl.ins,
                                   reason="scatter after its sample fills")

    nc.compile()
    return nc


def _compute_indices(coord_v, lows, highs, nmc, L_):
    """Replicates reference.py lines exactly (same jax ops on the default
    device) so the floor/log10 bin boundaries match bit-for-bit."""
    import jax.numpy as jnp

    cv = jnp.asarray(np.asarray(coord_v, dtype=np.float32))
    n = cv.shape[1] // 3
    v10 = cv.at[:, 2::3].set(jnp.log10(cv[:, 2::3]))
    lo = jnp.tile(jnp.asarray(np.asarray(lows, dtype=np.float32)), n)
    hi = jnp.tile(jnp.asarray(np.asarray(highs, dtype=np.float32)), n)
    coord_grid = (v10 - lo) / (hi - lo)
    tr = coord_grid.reshape(-1, 3)
    x_i = jnp.floor(tr[:, 0] * L_).astype(jnp.int32)
    y_i = jnp.floor(tr[:, 1] * L_).astype(jnp.int32)
    m_i = jnp.floor(tr[:, 2] * nmc).astype(jnp.int32)
    return (np.asarray(x_i), np.asarray(y_i), np.asarray(m_i))


def _prepare_in_maps(coord_v, lows, highs, nmc, L):
    nmc = int(nmc)
    L_ = int(L)
    x_i, y_i, m_i = _compute_indices(coord_v, lows, highs, nmc, L_)
    n_batch = coord_v.shape[0]
    n = coord_v.shape[1] // 3
    b_i = np.repeat(np.arange(n_batch, dtype=np.int64), n)

    # Flat element offsets (per core, local slab coordinates).
    flat_ones = ((b_i % BL) * SLAB + m_i.astype(np.int64) * PLANE
                 + y_i.astype(np.int64) * L_ + x_i.astype(np.int64))
    flat_z = flat_ones + HALF

    in_maps = []
    pts_per_core = BL * n  # 320
    for c in range(NCORES):
        sel = slice(c * pts_per_core, (c + 1) * pts_per_core)
        po = flat_ones[sel]
        pz = flat_z[sel]
        offs_np = np.zeros((128, N_SCATTER_COLS), dtype=np.int32)
        for j in range(7):   # cols 0-6: samples 4j..4j+3 (40 points)
            offs_np[0:40, j] = po[40 * j:40 * j + 40]
            offs_np[40:80, j] = pz[40 * j:40 * j + 40]
        # cols 7/8 rows 20-31 are dup-padding (vals there is 0.0, so they
        # must point at real ones-half cells)
        offs_np[0:20, 7] = po[280:300]   # samples 28-29
        offs_np[20:32, 7] = po[280]
        offs_np[32:52, 7] = pz[280:300]
        offs_np[0:20, 8] = po[300:320]   # samples 30-31
        offs_np[20:32, 8] = po[300]
        offs_np[32:52, 8] = pz[300:320]
        in_maps.append({"offs": offs_np})
    return in_maps


def _run(in_maps, **kwargs):
    if "nc" not in _CACHE:
        _CACHE["nc"] = _build_nc()
    nc = _CACHE["nc"]
    from concourse.bass_utils import run_bass_kernel_spmd
    return run_bass_kernel_spmd(nc, in_maps, core_ids=list(range(NCORES)),
                                **kwargs)


def kernel(coord_v, lows, highs, nmc, L):
    nmc = int(nmc)
    L_ = int(L)
    assert nmc == NMC and L_ == globals()["L"], (nmc, L_)

    in_maps = _prepare_in_maps(coord_v, lows, highs, nmc, L_)
    res = _run(in_maps)
    parts = [res.results[c]["out"].reshape(BL, 2 * NMC, L_, L_)
             for c in range(NCORES)]
    return np.concatenate(parts, axis=0)


# revision 19
# speedup vs baseline: 1.0357x; 1.0357x over previous
"""Trainium2 Bass kernel for nn_CustomParameterTransform (scatter_memory).

Reference semantics: coord_v [256, 30] holds 10 (x, y, mass) triplets per
sample. Each triplet maps to integer grid indices (x_i, y_i, m_i); a one-hot
volume z [B, 16, 128, 128] is scattered (z[b, m, y, x] = 1) and the output is
concat(1-z, z) over the channel axis -> [256, 32, 128, 128] f32 (512 MB).

Strategy (8 NeuronCores, batch-sharded, no cross-core comm):
  - The output is almost entirely constant: the first 16 channels are 1.0
    except at scatter points, the last 16 are 0.0 except at scatter points.
  - Per core (32 samples, 64 MB slab): fill the slab from constant SBUF
    tiles with large DMAs (write-only HBM traffic; ~425 GB/s sustained =
    16 SDMA engines x ~26.6 GB/s), then fix up the 640 scatter points with
    indirect (scatter) DMAs on the gpsimd/SWDGE queue.
  - Indices are computed on the host with the exact same jax ops as the
    reference (bit-identical floor/log10 behavior) and passed per-core as a
    [128, 6] int32 tensor of flat element offsets.

Trace-driven tuning (what each piece buys):
  - First memsets on the vector engine (gpsimd takes ~7 us to wake after
    the NEFF startup barrier; vector is ready at ~4.8 us) -> first fill
    at ~5.6 us instead of 8.4 us.
  - Fills run in ascending sample order; scatter columns for samples 0-29
    depend on fills that complete mid-stream, so those scatters fully
    overlap the fill phase. Only the last fill (samples 30-31) gates a
    scatter: a small 64-row column (fast SWDGE dispatch) -> post-fill tail
    ~2.5 us instead of ~4 us for a 128-row column gated on everything.
  - Light drain/barrier epilogue (see _light_drain_and_barrier). The
    remaining exit cost (~7 us: all-engine barrier + a NEFF-level sweep
    zeroing all 253 event semaphores + exit barrier) is emitted by the
    NEFF toolchain, not bass, and is not reachable from kernel code.
"""

import numpy as np

B = 256
NSRC = 10
NMC = 16
L = 128
NCORES = 8
BL = B // NCORES          # 32 samples per core
PLANE = L * L             # 16384
HALF = NMC * PLANE        # 262144 elements per half-slab
SLAB = 2 * HALF           # 524288 elements per sample
OUT_ELEMS = BL * SLAB     # 16777216 per core (64 MB)

N_SCATTER_COLS = 8        # col j covers samples 4j..4j+3

_CACHE = {}


def _build_nc():
    import concourse.bass as bass
    import concourse.tile as tile
    from concourse import bacc, mybir
    from concourse.tile_rust import add_dep_helper

    import types as _types
    from concourse.vector_clock import ScopedClock

    nc = bacc.Bacc("TRN2", target_bir_lowering=False, debug=False,
                   num_devices=NCORES)

    def _light_drain_and_barrier(self, tick_clock, wait_clock):
        """Replaces TileContext._drain_and_barrier for this kernel. The
        stock epilogue is drain + two all-engine EVSEM butterfly barriers
        around the sem clear. Requirements at kernel end are: (1) all DMA
        completions observed, (2) sems cleared for NEFF re-execution,
        (3) the clear happens after every engine's last sem use. (1) is
        the sync drain's global-clock waits; (3) is a counting-sem join
        (sync arrives only after the drain, so join>=4 implies all DMA
        done); (2) is the ranged clear. The second barrier is
        unnecessary: a re-execution cannot start until every engine --
        including the clearing gpsimd -- has ended."""
        nc_ = self.nc
        drain_inst = nc_.sync.drain()
        wait_clock.add_sem_waits(
            drain_inst.ins, ScopedClock({None: tick_clock.global_clock}))
        join = nc_.alloc_semaphore("tail_join")
        for eng in nc_.engines.values():
            if eng is not nc_.gpsimd:
                eng.sem_inc(join, 1)
        n_other = len(nc_.engines) - 1
        nc_.gpsimd.wait_ge(join, n_other)
        popped = nc_._tile_sem_poison_stack.pop()
        assert popped is self._sem_poison
        sems = list(self.sems.allocated().values())
        nc_.clear_and_free_semaphores(sems + [join])

    offs = nc.dram_tensor("offs", [128, N_SCATTER_COLS], mybir.dt.int32,
                          kind="ExternalInput").ap()
    out = nc.dram_tensor("out", [OUT_ELEMS], mybir.dt.float32,
                         kind="ExternalOutput").ap()

    with tile.TileContext(nc) as tc:
        tc._drain_and_barrier = _types.MethodType(_light_drain_and_barrier, tc)
        with tc.tile_pool(name="src", bufs=1) as src_pool, \
             tc.tile_pool(name="small", bufs=1) as small_pool:
            # Constant source tiles. Memset cost scales with the free-dim
            # cols (128 lanes run in parallel), so big tiles are split
            # column-wise between vector and gpsimd. Vector handles
            # everything needed early (it wakes ~2 us before gpsimd).
            # Starters: tiny tiles memset by gpsimd (which enters the tile
            # block at ~6.4 us, a hair before vector's first memset lands)
            # so each queue's very first 256 KB fill can launch ~0.7 us
            # earlier than the minis allow.
            starter_ones = src_pool.tile([128, 512], mybir.dt.float32)
            starter_zeros = src_pool.tile([128, 512], mybir.dt.float32)
            nc.gpsimd.memset(starter_ones[:, :], 1.0)
            nc.gpsimd.memset(starter_zeros[:, :], 0.0)
            ones_mini = src_pool.tile([128, 1024], mybir.dt.float32)
            zeros_mini = src_pool.tile([128, 1024], mybir.dt.float32)
            nc.vector.memset(ones_mini[:, :], 1.0)
            nc.vector.memset(zeros_mini[:, :], 0.0)
            # combo: one full slab ([128, 4096]; DMA iterates partition-
            # major, so partitions 0-63 are the ones half, 64-127 zeros).
            combo_t = src_pool.tile([128, 4096], mybir.dt.float32)
            nc.vector.memset(combo_t[0:64, 0:2048], 1.0)
            nc.vector.memset(combo_t[64:128, 0:2048], 0.0)
            nc.gpsimd.memset(combo_t[0:64, 2048:4096], 1.0)
            nc.gpsimd.memset(combo_t[64:128, 2048:4096], 0.0)
            # mega: two slabs ([128, 8192]; slab = 64 partitions, ones iff
            # p%64 < 32). Feeds samples 6-29 as 4 MB fills.
            mega_t = src_pool.tile([128, 8192], mybir.dt.float32)
            for lo, hi, v in ((0, 32, 1.0), (32, 64, 0.0),
                              (64, 96, 1.0), (96, 128, 0.0)):
                nc.vector.memset(mega_t[lo:hi, 0:4096], v)
                nc.gpsimd.memset(mega_t[lo:hi, 4096:8192], v)

            # Scatter offsets: [128, 9] int32 flat element indices.
            # Columns are ALIGNED TO FILL ORDER so each column's fill deps
            # complete early in the stream and the scatter hides under the
            # remaining fills. (A column spanning 13 samples would gate on
            # the latest of 7 fills -> a serialized gpsimd dispatch pileup
            # at the stream tail, ~+6 us.)
            #   col j in 0..7: samples 4j..4j+3; rows 0-39 ones-half
            #                  writes (0.0), rows 40-79 z-half writes (1.0)
            # col 7 (samples 28-31) is the only one gated near the stream
            # end; engine-progress skew makes the second-to-last fill
            # complete ~at the end anyway, so splitting it further just
            # adds a serialized scatter (they share the narrow out AP, so
            # Tile WAW-chains them on completion).
            # offs loads on the scalar HWDGE queue before its fills (no
            # deps, so Tile lets it lead); vals memsets ride on vector
            # after its big memsets. Engine ops must start at a partition
            # multiple of 32, hence the overwrite at rows 32:40.
            offs_t = small_pool.tile([128, N_SCATTER_COLS], mybir.dt.int32)
            vals_t = small_pool.tile([128, N_SCATTER_COLS], mybir.dt.float32)
            nc.vector.memset(vals_t[0:32, :], 0.0)
            nc.vector.memset(vals_t[32:64, :], 1.0)
            nc.vector.memset(vals_t[32:40, :], 0.0)
            nc.vector.memset(vals_t[64:96, :], 1.0)

            MINI = 131072  # elements per mini fill (512 KB)
            ones_fills = {}   # sample -> list of fills covering its ones half
            zeros_fills = {}  # sample -> list of fills covering its zeros half
            fill_seq = {"sync": 0, "scalar": 0}

            # Samples 0-1 from starters + minis (ready first). Each half
            # (1 MB) = starter 256 KB + mini 512 KB + mini-slice 256 KB.
            STRT = 65536
            for s in (0, 1):
                e_ones = nc.sync if s == 0 else nc.scalar
                e_zeros = nc.scalar if s == 0 else nc.sync
                base = s * SLAB
                ones_fills[s] = [
                    e_ones.dma_start(out[base:base + STRT],
                                     starter_ones[:, :]),
                    e_ones.dma_start(out[base + STRT:base + STRT + MINI],
                                     ones_mini[:, :]),
                    e_ones.dma_start(out[base + STRT + MINI:base + HALF],
                                     ones_mini[:, 0:512]),
                ]
                zbase = base + HALF
                zeros_fills[s] = [
                    e_zeros.dma_start(out[zbase:zbase + STRT],
                                      starter_zeros[:, :]),
                    e_zeros.dma_start(out[zbase + STRT:zbase + STRT + MINI],
                                      zeros_mini[:, :]),
                    e_zeros.dma_start(out[zbase + STRT + MINI:zbase + HALF],
                                      zeros_mini[:, 0:512]),
                ]
            # offs load sits on the scalar queue here: after the first
            # mini fills (so it doesn't delay the stream start), done by
            # ~8 us, way before the first scatter needs it (~70 us).
            offs_fill = nc.scalar.dma_start(offs_t[:, :], offs[:, :])
            # Samples 2-5 from combo (2 MB fills).
            for s in range(2, 6):
                eng = nc.sync if s % 2 == 0 else nc.scalar
                f = eng.dma_start(out[s * SLAB:(s + 1) * SLAB], combo_t[:, :])
                ones_fills[s] = [f]
                zeros_fills[s] = [f]
            # Samples 6-29 from mega (4 MB pair fills), ascending, so the
            # scatter columns' fill deps complete early in the stream.
            for s in range(6, 30, 2):
                eng = nc.sync if (s // 2) % 2 == 0 else nc.scalar
                f = eng.dma_start(out[s * SLAB:(s + 2) * SLAB], mega_t[:, :])
                for ss in (s, s + 1):
                    ones_fills[ss] = [f]
                    zeros_fills[ss] = [f]
            # Samples 30-31 last, one 2 MB fill per queue (balances both
            # queues at 32 MB) from mega's two slab halves.
            f30 = nc.sync.dma_start(out[30 * SLAB:31 * SLAB], mega_t[0:64, :])
            f31 = nc.scalar.dma_start(out[31 * SLAB:32 * SLAB],
                                      mega_t[64:128, :])
            ones_fills[30] = [f30]
            zeros_fills[30] = [f30]
            ones_fills[31] = [f31]
            zeros_fills[31] = [f31]

            # Which sample-fills each scatter column touches.
            def deps(lo, hi):
                seen = {}
                for s in range(lo, hi):
                    for f in ones_fills[s] + zeros_fills[s]:
                        seen[id(f)] = f
                return list(seen.values())
            col_specs = [(slice(0, 80), deps(4 * j, 4 * j + 4))
                         for j in range(8)]

            # Narrow declared out AP ([1, 1] at offset 0, required by the
            # indirect API): the real write addresses come from the offset
            # tensor; a full-tensor AP would make Tile serialize every
            # scatter behind every fill (WAW), and the explicit col_deps
            # edges below provide the true ordering. (The shared AP does
            # WAW-chain the scatters behind each other, which is fine:
            # each chain link completes long before the next column's
            # fill deps, except the last -- and there is only one
            # tail-gated column.)
            out2d = out[0:1].unsqueeze(1)
            for j, (rows, fl_deps) in enumerate(col_specs):
                sc = nc.gpsimd.indirect_dma_start(
                    out=out2d,
                    out_offset=bass.IndirectOffsetOnAxis(
                        ap=offs_t[rows, j:j + 1], axis=0),
                    in_=vals_t[rows, j:j + 1],
                    in_offset=None,
                )
                for fl in fl_deps:
                    add_dep_helper(sc.ins, fl.ins,
                                   reason="scatter after its sample fills")

    nc.compile()
    return nc


def _compute_indices(coord_v, lows, highs, nmc, L_):
    """Replicates reference.py lines exactly (same jax ops on the default
    device) so the floor/log10 bin boundaries match bit-for-bit."""
    import jax.numpy as jnp

    cv = jnp.asarray(np.asarray(coord_v, dtype=np.float32))
    n = cv.shape[1] // 3
    v10 = cv.at[:, 2::3].set(jnp.log10(cv[:, 2::3]))
    lo = jnp.tile(jnp.asarray(np.asarray(lows, dtype=np.float32)), n)
    hi = jnp.tile(jnp.asarray(np.asarray(highs, dtype=np.float32)), n)
    coord_grid = (v10 - lo) / (hi - lo)
    tr = coord_grid.reshape(-1, 3)
    x_i = jnp.floor(tr[:, 0] * L_).astype(jnp.int32)
    y_i = jnp.floor(tr[:, 1] * L_).astype(jnp.int32)
    m_i = jnp.floor(tr[:, 2] * nmc).astype(jnp.int32)
    return (np.asarray(x_i), np.asarray(y_i), np.asarray(m_i))


def _prepare_in_maps(coord_v, lows, highs, nmc, L):
    nmc = int(nmc)
    L_ = int(L)
    x_i, y_i, m_i = _compute_indices(coord_v, lows, highs, nmc, L_)
    n_batch = coord_v.shape[0]
    n = coord_v.shape[1] // 3
    b_i = np.repeat(np.arange(n_batch, dtype=np.int64), n)

    # Flat element offsets (per core, local slab coordinates).
    flat_ones = ((b_i % BL) * SLAB + m_i.astype(np.int64) * PLANE
                 + y_i.astype(np.int64) * L_ + x_i.astype(np.int64))
    flat_z = flat_ones + HALF

    in_maps = []
    pts_per_core = BL * n  # 320
    for c in range(NCORES):
        sel = slice(c * pts_per_core, (c + 1) * pts_per_core)
        po = flat_ones[sel]
        pz = flat_z[sel]
        offs_np = np.zeros((128, N_SCATTER_COLS), dtype=np.int32)
        for j in range(8):   # col j: samples 4j..4j+3 (40 points)
            offs_np[0:40, j] = po[40 * j:40 * j + 40]
            offs_np[40:80, j] = pz[40 * j:40 * j + 40]
        in_maps.append({"offs": offs_np})
    return in_maps


def _run(in_maps, **kwargs):
    if "nc" not in _CACHE:
        _CACHE["nc"] = _build_nc()
    nc = _CACHE["nc"]
    from concourse.bass_utils import run_bass_kernel_spmd
    return run_bass_kernel_spmd(nc, in_maps, core_ids=list(range(NCORES)),
                                **kwargs)


def kernel(coord_v, lows, highs, nmc, L):
    nmc = int(nmc)
    L_ = int(L)
    assert nmc == NMC and L_ == globals()["L"], (nmc, L_)

    in_maps = _prepare_in_maps(coord_v, lows, highs, nmc, L_)
    res = _run(in_maps)
    parts = [res.results[c]["out"].reshape(BL, 2 * NMC, L_, L_)
             for c in range(NCORES)]
    return np.concatenate(parts, axis=0)


# revision 21
# speedup vs baseline: 1.0888x; 1.0513x over previous
"""Trainium2 Bass kernel for nn_CustomParameterTransform (scatter_memory).

Reference semantics: coord_v [256, 30] holds 10 (x, y, mass) triplets per
sample. Each triplet maps to integer grid indices (x_i, y_i, m_i); a one-hot
volume z [B, 16, 128, 128] is scattered (z[b, m, y, x] = 1) and the output is
concat(1-z, z) over the channel axis -> [256, 32, 128, 128] f32 (512 MB).

Strategy (8 NeuronCores, batch-sharded, no cross-core comm):
  - The output is almost entirely constant: the first 16 channels are 1.0
    except at scatter points, the last 16 are 0.0 except at scatter points.
  - Per core (32 samples, 64 MB slab): fill the slab from constant SBUF
    tiles with large DMAs (write-only HBM traffic; ~425 GB/s sustained =
    16 SDMA engines x ~26.6 GB/s, the SBUF-AXI port limit), then fix up
    the 640 scatter points with indirect (scatter) DMAs on gpsimd/SWDGE.
  - Indices are computed on the host with the exact same jax ops as the
    reference (bit-identical floor/log10 behavior) and passed per-core as
    a [128, 8] int32 tensor of flat element offsets.

Trace-driven structure (each piece measured):
  - Both HWDGE queues (sync + scalar) carry 32 MB each and transition
    descriptor sizes IN LOCKSTEP (512 KB mini fills with 4 KB descs ->
    2 MB combo fills with 16 KB descs -> 4 MB mega fills with 32 KB
    descs -> 2 MB combo-sourced final fills). Windows where the two
    queues run different descriptor sizes measurably degrade all
    engines (~2x packet times); keeping them aligned sustains the
    ~425 GB/s ceiling.
  - First memsets on the vector engine (gpsimd wakes a little later) ->
    first fill at ~7.7 us instead of 8.4.
  - Scatter columns are aligned to the fill order: col j covers samples
    4j..4j+3, so cols 0-6 gate on fills that complete early/mid-stream
    and fully hide. Only col 7 (samples 28-31, the final fills) runs
    after the stream: one small scatter, ~2.6 us.
  - Light drain/barrier epilogue (see _light_drain_and_barrier). The
    remaining ~6 us exit cost (a NEFF-level sweep zeroing all event
    semaphores plus two exit barriers) is emitted by the NEFF toolchain
    downstream of bass and is not reachable from kernel code.
"""

import numpy as np

B = 256
NSRC = 10
NMC = 16
L = 128
NCORES = 8
BL = B // NCORES          # 32 samples per core
PLANE = L * L             # 16384
HALF = NMC * PLANE        # 262144 elements per half-slab
SLAB = 2 * HALF           # 524288 elements per sample
OUT_ELEMS = BL * SLAB     # 16777216 per core (64 MB)

N_SCATTER_COLS = 8        # col j covers samples 4j..4j+3

_CACHE = {}


def _build_nc():
    import concourse.bass as bass
    import concourse.tile as tile
    from concourse import bacc, mybir
    from concourse.tile_rust import add_dep_helper

    import types as _types
    from concourse.vector_clock import ScopedClock

    nc = bacc.Bacc("TRN2", target_bir_lowering=False, debug=False,
                   num_devices=NCORES)

    def _light_drain_and_barrier(self, tick_clock, wait_clock):
        """Replaces TileContext._drain_and_barrier for this kernel. The
        stock epilogue is drain + two all-engine EVSEM butterfly barriers
        around the sem clear. Requirements at kernel end are: (1) all DMA
        completions observed, (2) sems cleared for NEFF re-execution,
        (3) the clear happens after every engine's last sem use. (1) is
        the sync drain's global-clock waits; (3) is a counting-sem join
        (sync arrives only after the drain, so join>=4 implies all DMA
        done); (2) is the ranged clear. The second barrier is
        unnecessary: a re-execution cannot start until every engine --
        including the clearing gpsimd -- has ended."""
        nc_ = self.nc
        drain_inst = nc_.sync.drain()
        wait_clock.add_sem_waits(
            drain_inst.ins, ScopedClock({None: tick_clock.global_clock}))
        join = nc_.alloc_semaphore("tail_join")
        for eng in nc_.engines.values():
            if eng is not nc_.gpsimd:
                eng.sem_inc(join, 1)
        n_other = len(nc_.engines) - 1
        nc_.gpsimd.wait_ge(join, n_other)
        popped = nc_._tile_sem_poison_stack.pop()
        assert popped is self._sem_poison
        sems = list(self.sems.allocated().values())
        nc_.clear_and_free_semaphores(sems + [join])

    offs = nc.dram_tensor("offs", [128, N_SCATTER_COLS], mybir.dt.int32,
                          kind="ExternalInput").ap()
    out = nc.dram_tensor("out", [OUT_ELEMS], mybir.dt.float32,
                         kind="ExternalOutput").ap()

    with tile.TileContext(nc) as tc:
        tc._drain_and_barrier = _types.MethodType(_light_drain_and_barrier, tc)
        with tc.tile_pool(name="src", bufs=1) as src_pool, \
             tc.tile_pool(name="small", bufs=1) as small_pool:
            # Constant source tiles. Memset cost scales with the free-dim
            # cols (128 lanes run in parallel), so big tiles are split
            # column-wise between vector and gpsimd; the minis go to
            # vector alone because it reaches its first memset slightly
            # before gpsimd and the first fills wait on them.
            ones_mini = src_pool.tile([128, 1024], mybir.dt.float32)
            zeros_mini = src_pool.tile([128, 1024], mybir.dt.float32)
            nc.vector.memset(ones_mini[:, :], 1.0)
            nc.vector.memset(zeros_mini[:, :], 0.0)
            # combo: one full slab ([128, 4096]; DMA iterates partition-
            # major, so partitions 0-63 are the ones half, 64-127 zeros).
            # Feeds samples 2-5 early and samples 30-31 at the very end.
            combo_t = src_pool.tile([128, 4096], mybir.dt.float32)
            nc.vector.memset(combo_t[0:64, 0:2048], 1.0)
            nc.vector.memset(combo_t[64:128, 0:2048], 0.0)
            nc.gpsimd.memset(combo_t[0:64, 2048:4096], 1.0)
            nc.gpsimd.memset(combo_t[64:128, 2048:4096], 0.0)
            # mega: two slabs ([128, 8192]; slab = 64 partitions, ones iff
            # p%64 < 32). Feeds samples 6-29 as 4 MB pair fills.
            mega_t = src_pool.tile([128, 8192], mybir.dt.float32)
            for lo, hi, v in ((0, 32, 1.0), (32, 64, 0.0),
                              (64, 96, 1.0), (96, 128, 0.0)):
                nc.vector.memset(mega_t[lo:hi, 0:4096], v)
                nc.gpsimd.memset(mega_t[lo:hi, 4096:8192], v)

            # Scatter offsets: [128, 8] int32 flat element indices.
            # Columns are ALIGNED TO FILL ORDER: col j covers samples
            # 4j..4j+3 (rows 0-39 ones-half writes of 0.0, rows 40-79
            # z-half writes of 1.0), so each column's fill deps complete
            # early relative to the stream end -- except col 7, whose
            # samples are filled last by design (one small tail scatter).
            # (A column spanning 13 samples would gate on the latest of 7
            # fills -> a serialized gpsimd dispatch pileup at the tail.)
            # offs loads on the gpsimd queue after its memsets (~31 us,
            # before the first column needs it); vals memsets ride on
            # vector after its big memsets. Engine ops must start at a
            # partition multiple of 32, hence the overwrite at rows 32:40.
            offs_t = small_pool.tile([128, N_SCATTER_COLS], mybir.dt.int32)
            nc.gpsimd.dma_start(offs_t[:, :], offs[:, :])
            vals_t = small_pool.tile([128, N_SCATTER_COLS], mybir.dt.float32)
            nc.vector.memset(vals_t[0:32, :], 0.0)
            nc.vector.memset(vals_t[32:64, :], 1.0)
            nc.vector.memset(vals_t[32:40, :], 0.0)
            nc.vector.memset(vals_t[64:96, :], 1.0)

            MINI = 131072  # elements per mini fill (512 KB)
            ones_fills = {}   # sample -> list of fills covering its ones half
            zeros_fills = {}  # sample -> list of fills covering its zeros half

            # Samples 0-1 from the minis (ready first; 4 KB descriptors).
            for s in (0, 1):
                e_ones = nc.sync if s == 0 else nc.scalar
                e_zeros = nc.scalar if s == 0 else nc.sync
                ones_fills[s] = [
                    e_ones.dma_start(
                        out[s * SLAB + k * MINI:s * SLAB + (k + 1) * MINI],
                        ones_mini[:, :])
                    for k in range(2)]
                zeros_fills[s] = [
                    e_zeros.dma_start(
                        out[s * SLAB + HALF + k * MINI:
                            s * SLAB + HALF + (k + 1) * MINI],
                        zeros_mini[:, :])
                    for k in range(2)]
            # Samples 2-5 from combo (2 MB fills, 16 KB descriptors).
            for s in range(2, 6):
                eng = nc.sync if s % 2 == 0 else nc.scalar
                f = eng.dma_start(out[s * SLAB:(s + 1) * SLAB], combo_t[:, :])
                ones_fills[s] = [f]
                zeros_fills[s] = [f]
            # Samples 6-29 from mega (4 MB pair fills, 32 KB descriptors),
            # ascending so the scatter columns' deps complete early.
            for s in range(6, 30, 2):
                eng = nc.sync if (s // 2) % 2 == 0 else nc.scalar
                f = eng.dma_start(out[s * SLAB:(s + 2) * SLAB], mega_t[:, :])
                for ss in (s, s + 1):
                    ones_fills[ss] = [f]
                    zeros_fills[ss] = [f]
            # Samples 30-31 last, one 2 MB combo fill per queue (balances
            # both queues at 32 MB; full-128-partition source keeps the
            # descriptor profile identical to the earlier combo fills).
            f30 = nc.sync.dma_start(out[30 * SLAB:31 * SLAB], combo_t[:, :])
            f31 = nc.scalar.dma_start(out[31 * SLAB:32 * SLAB], combo_t[:, :])
            ones_fills[30] = [f30]
            zeros_fills[30] = [f30]
            ones_fills[31] = [f31]
            zeros_fills[31] = [f31]

            # Which sample-fills each scatter column touches.
            def deps(lo, hi):
                seen = {}
                for s in range(lo, hi):
                    for f in ones_fills[s] + zeros_fills[s]:
                        seen[id(f)] = f
                return list(seen.values())
            col_specs = [(slice(0, 80), deps(4 * j, 4 * j + 4))
                         for j in range(8)]

            # Narrow declared out AP ([1, 1] at offset 0, required by the
            # indirect API): the real write addresses come from the offset
            # tensor; a full-tensor AP would make Tile serialize every
            # scatter behind every fill (WAW), and the explicit col_deps
            # edges below provide the true ordering. (The shared AP does
            # WAW-chain the scatters behind each other, which is fine:
            # each chain link completes long before the next column's
            # fill deps, except the last -- and there is only one
            # tail-gated column.)
            out2d = out[0:1].unsqueeze(1)
            for j, (rows, fl_deps) in enumerate(col_specs):
                sc = nc.gpsimd.indirect_dma_start(
                    out=out2d,
                    out_offset=bass.IndirectOffsetOnAxis(
                        ap=offs_t[rows, j:j + 1], axis=0),
                    in_=vals_t[rows, j:j + 1],
                    in_offset=None,
                )
                for fl in fl_deps:
                    add_dep_helper(sc.ins, fl.ins,
                                   reason="scatter after its sample fills")

    nc.compile()
    return nc


def _compute_indices(coord_v, lows, highs, nmc, L_):
    """Replicates reference.py lines exactly (same jax ops on the default
    device) so the floor/log10 bin boundaries match bit-for-bit."""
    import jax.numpy as jnp

    cv = jnp.asarray(np.asarray(coord_v, dtype=np.float32))
    n = cv.shape[1] // 3
    v10 = cv.at[:, 2::3].set(jnp.log10(cv[:, 2::3]))
    lo = jnp.tile(jnp.asarray(np.asarray(lows, dtype=np.float32)), n)
    hi = jnp.tile(jnp.asarray(np.asarray(highs, dtype=np.float32)), n)
    coord_grid = (v10 - lo) / (hi - lo)
    tr = coord_grid.reshape(-1, 3)
    x_i = jnp.floor(tr[:, 0] * L_).astype(jnp.int32)
    y_i = jnp.floor(tr[:, 1] * L_).astype(jnp.int32)
    m_i = jnp.floor(tr[:, 2] * nmc).astype(jnp.int32)
    return (np.asarray(x_i), np.asarray(y_i), np.asarray(m_i))


def _prepare_in_maps(coord_v, lows, highs, nmc, L):
    nmc = int(nmc)
    L_ = int(L)
    x_i, y_i, m_i = _compute_indices(coord_v, lows, highs, nmc, L_)
    n_batch = coord_v.shape[0]
    n = coord_v.shape[1] // 3
    b_i = np.repeat(np.arange(n_batch, dtype=np.int64), n)

    # Flat element offsets (per core, local slab coordinates).
    flat_ones = ((b_i % BL) * SLAB + m_i.astype(np.int64) * PLANE
                 + y_i.astype(np.int64) * L_ + x_i.astype(np.int64))
    flat_z = flat_ones + HALF

    in_maps = []
    pts_per_core = BL * n  # 320
    for c in range(NCORES):
        sel = slice(c * pts_per_core, (c + 1) * pts_per_core)
        po = flat_ones[sel]
        pz = flat_z[sel]
        offs_np = np.zeros((128, N_SCATTER_COLS), dtype=np.int32)
        for j in range(8):   # col j: samples 4j..4j+3 (40 points)
            offs_np[0:40, j] = po[40 * j:40 * j + 40]
            offs_np[40:80, j] = pz[40 * j:40 * j + 40]
        in_maps.append({"offs": offs_np})
    return in_maps


def _run(in_maps, **kwargs):
    if "nc" not in _CACHE:
        _CACHE["nc"] = _build_nc()
    nc = _CACHE["nc"]
    from concourse.bass_utils import run_bass_kernel_spmd
    return run_bass_kernel_spmd(nc, in_maps, core_ids=list(range(NCORES)),
                                **kwargs)


def kernel(coord_v, lows, highs, nmc, L):
    nmc = int(nmc)
    L_ = int(L)
    assert nmc == NMC and L_ == globals()["L"], (nmc, L_)

    in_maps = _prepare_in_maps(coord_v, lows, highs, nmc, L_)
    res = _run(in_maps)
    parts = [res.results[c]["out"].reshape(BL, 2 * NMC, L_, L_)
             for c in range(NCORES)]
    return np.concatenate(parts, axis=0)
